# revision 46
# baseline (speedup 1.0000x reference)
"""Multi-head attention (B=2, S=2048, D=1024, H=16) on 8 NeuronCores.

Sharding: core c -> (batch b = c // 4, head-group g = c % 4). Each core
computes 4 heads of one batch plus the partial output projection for its
head-group's rows of Wo; the host sums the 4 partials per batch and adds bo.

Key-side compaction: masked key positions (True in `mask`) contribute
exactly zero attention weight, so the host drops them before sharding —
key/value inputs, K/V projections, score matmuls, the exp() pass and the
ctx matmuls all shrink by the masked fraction. The compacted length is
padded to a multiple of 128 with zero-columns whose mask bias (-60, applied
inside the exp activation) keeps their contribution at ~1e-26.

Layout strategy (per core):
  - Inputs are host-transposed: x^T [D, S*] so projections run with W as the
    stationary operand and x^T as the moving operand. Weights are
    host-prearranged into the [128, kt, n] SBUF layout so their DMA is one
    contiguous descriptor per partition row.
  - Q/K projections run in fp8-e4m3 with DoubleRow perf mode: x and W are
    quantized host-side (W scaled by 16 to stay in e4m3 normal range; the
    1/16 folds into the PSUM evacuation), and each matmul contracts TWO
    128-row D-tiles at once, halving the projection pass count. Errors of
    the fp8 quantization average out in the length-1024 dot products
    (~0.2% on q/k, ~0.6% on logits). Q^T/K^T themselves stay bf16.
  - Scores are computed TRANSPOSED: S^T[k, q] = K Q^T, so the key-position
    (padding) mask is per-PARTITION and folds into the single exp()
    activation as a bias AP, along with the 1/sqrt(dk) scale. One exp per
    [128, 2, w] PSUM tile covers both heads of a pair (the two heads' score
    matmuls run concurrently via PE row-tiling, K=64 each; the per-head row
    stride is padded to 512 so both destinations stay PSUM-bank-aligned).
  - V is produced in natural [S*, dv] layout with a ones-column per head
    (bias folded via an augmented contraction row), so the ctx matmul
    ctx^T = [V_h | 1]^T @ P^T also yields the softmax denominator as row 64.
  - Normalization: the denominator rows are copied to SBUF, reciprocal'd
    with the custom-DVE fast approximation (~18-bit, one instruction), cast
    to bf16, broadcast across partitions with two K=1 outer-product matmuls,
    then DVE multiplies. Pipelined one unit behind the matmul blocks.
  - Pipeline shape: K proj first, then 10 attention units (chunk, head-pair)
    with scores/exp running `lag` kt-slots ahead of ctx. ALL auxiliary PE
    work — V-proj m-tiles (unit 0), the previous unit's norm broadcast,
    O-proj m-tiles, Q-proj pair-tiles — is injected one item per kt slot
    INSIDE the attention stream, so the score matmuls (and the ACT exp
    stream they feed) never see a multi-us convoy of other PE work and the
    PE stays dense enough to hold the HAM clock gate at full rate.
  - Tail: the last q-chunk is split 384+128 and O-projection m-tiles are
    spread on an eligibility-driven schedule, so after the final unit's
    normalization only one 128-row m-tile remains.

Compute dtype (env KDT): "bf16" (default) uses bfloat16 matmul operands
(~5e-3 rel err, 1 cyc/row PE + half the DMA of f32); "f32r" keeps float32r
operands (~2e-4 rel err but ~2x slower matmuls). KF8=0 disables the fp8
projection path.
"""

import os
from contextlib import ExitStack

import numpy as np

import concourse.bacc as bacc
import concourse.mybir as mybir
import concourse.tile as tile

F32 = mybir.dt.float32
F32R = mybir.dt.float32r
BF16 = mybir.dt.bfloat16
FP8 = mybir.dt.float8e4
AF = mybir.ActivationFunctionType
ALU = mybir.AluOpType
MPD = mybir.MatmulPerfMode.DoubleRow

B, S, D = 2, 2048, 1024
H, DK = 16, 64
G = 4                    # head-groups (tensor parallel)
HPG = H // G             # 4 heads per group
DG = HPG * DK            # 256 head dims per group
NCORES = 8
MASK_NEG = -60.0         # additive post-scale bias for padded key positions
SCALE = 0.125            # 1/sqrt(dk)
WSC = 16.0               # host-side W_q/W_k scale for fp8 normal range

KT_D = D // 128          # 8 contraction tiles for projections
KP = KT_D // 2           # 4 DoubleRow kt-pairs
NT = DG // 128           # 2 partition-tiles of qT/kT/cT (one head-pair each)
QC = 512                 # q projection chunk (matmul moving dim)
NQC = S // QC            # 4
# attention-unit q chunks: last 512 chunk split 384+128 so the tail after
# the final normalization is a single O-proj m-tile
QCHUNKS = [(0, 512), (512, 512), (1024, 512), (1536, 384), (1920, 128)]
VW = HPG * (DK + 1)      # 260: V width incl. per-head ones column

KDT = os.environ.get("KDT", "bf16")
KF8 = os.environ.get("KF8", "1") == "1" and KDT == "bf16"


def _dt():
    return BF16 if KDT == "bf16" else F32R


def _np_dt():
    import ml_dtypes

    return ml_dtypes.bfloat16 if KDT == "bf16" else np.float32


def build_bass(ktk):
    """Build the SPMD program for `ktk` 128-wide key tiles (SK = 128*ktk)."""
    SK = 128 * ktk
    kchunks = [(n0, min(QC, SK - n0)) for n0 in range(0, SK, QC)]
    cdt = _dt()
    idt = FP8 if KF8 else cdt      # x_q/x_k + W_q/W_k dtype
    qsc = 1.0 / WSC if KF8 else 1.0

    nc = bacc.Bacc(None, target_bir_lowering=False, debug=False)

    xq = nc.dram_tensor("xq", [D, S], idt, kind="ExternalInput")
    xk = nc.dram_tensor("xk", [D, SK], idt, kind="ExternalInput")
    xv = nc.dram_tensor("xv", [D, SK], cdt, kind="ExternalInput")
    # weights pre-arranged host-side into the SBUF tile layout (contiguous
    # per-partition rows -> few large DMA descriptors)
    wq = nc.dram_tensor("wq", [128, KT_D, DG], idt, kind="ExternalInput")
    wk = nc.dram_tensor("wk", [128, KT_D, DG], idt, kind="ExternalInput")
    wv = nc.dram_tensor("wv", [128, KT_D, VW], cdt, kind="ExternalInput")
    wvb = nc.dram_tensor("wvb", [1, VW], cdt, kind="ExternalInput")
    wo = nc.dram_tensor("wo", [128, NT, D], cdt, kind="ExternalInput")
    bq = nc.dram_tensor("bq", [128, NT], F32, kind="ExternalInput")
    bk = nc.dram_tensor("bk", [128, NT], F32, kind="ExternalInput")
    mb = nc.dram_tensor("mb", [128, ktk], F32, kind="ExternalInput")
    cstc = nc.dram_tensor("cstc", [3, 128], cdt, kind="ExternalInput")
    # bf16 partials: the host sums 4 head-group partials per batch in f32,
    # so the extra rounding is ~0.1% while output DMA bytes halve
    odt = BF16 if KDT == "bf16" else F32
    out = nc.dram_tensor("out", [S, D], odt, kind="ExternalOutput")

    with tile.TileContext(nc) as tc, ExitStack() as ctx:
        consts = ctx.enter_context(tc.tile_pool(name="consts", bufs=1))
        resid = ctx.enter_context(tc.tile_pool(name="resid", bufs=1))
        stream = ctx.enter_context(tc.tile_pool(name="stream", bufs=4))
        vstream = ctx.enter_context(tc.tile_pool(name="vstream", bufs=8))
        ptp = ctx.enter_context(tc.tile_pool(name="ptp", bufs=10 if ktk <= 12 else 4))
        smalls = ctx.enter_context(tc.tile_pool(name="smalls", bufs=3 if ktk <= 12 else 2))
        obp = ctx.enter_context(tc.tile_pool(name="obp", bufs=3))

        # ---------------- constants / weights declarations ----------------
        wq_s = consts.tile([128, KT_D, DG], idt, tag="wq_s", name="wq_s")
        bq_s = consts.tile([128, NT], F32, tag="bq_s", name="bq_s")
        wk_s = consts.tile([128, KT_D, DG], idt, tag="wk_s", name="wk_s")
        bk_s = consts.tile([128, NT], F32, tag="bk_s", name="bk_s")
        wv_s = consts.tile([128, KT_D, VW], cdt, tag="wv_s", name="wv_s")
        wv_b = consts.tile([1, VW], cdt, tag="wv_b", name="wv_b")
        wo_s = consts.tile([128, NT, D], cdt, tag="wo_s", name="wo_s")
        mb_s = consts.tile([128, ktk], F32, tag="mb_s", name="mb_s")
        # Constant rows (all-ones, head-A selector, head-B selector) come
        # from tiny DRAM inputs — memset can't write float32r tiles.
        ones1 = consts.tile([1, 128], cdt, tag="ones1", name="ones1")
        onesA = consts.tile([1, 128], cdt, tag="onesA", name="onesA")
        onesB = consts.tile([1, 128], cdt, tag="onesB", name="onesB")

        # ---------------- input stream prefetch ----------------
        # DMA issue order sets time-to-first-exp (the ACT exp stream is the
        # kernel's long pole): wk+xk first (K proj is the first PE work),
        # then wq + the q-chunk-0 slice of xq (first attention unit), xv
        # (V must finish right before the first ctx matmuls), then the rest
        # of xq streaming in under the attention phase. The x streams land
        # in kt-PAIR tiles [128, 2, w] so the fp8 DoubleRow matmuls can
        # address both contraction tiles in one AP.
        nc.sync.dma_start(out=wk_s, in_=wk[:])
        nc.sync.dma_start(out=bk_s, in_=bk[:])
        nc.sync.dma_start(out=mb_s, in_=mb[:])
        xk_t, xv_t, xq0_t, xqr_t = [], [], [], []
        for kp in range(KP):
            t_ = stream.tile([128, 2, SK], idt, tag="xk", name="xk_s")
            for j in range(2):
                nc.sync.dma_start(
                    out=t_[:, j, :],
                    in_=xk[(2 * kp + j) * 128 : (2 * kp + j + 1) * 128, :],
                )
            xk_t.append(t_)
        nc.sync.dma_start(out=wq_s, in_=wq[:])
        nc.sync.dma_start(out=bq_s, in_=bq[:])
        for kp in range(KP):
            t_ = stream.tile([128, 2, QC], idt, tag="xq0", name="xq0_s")
            for j in range(2):
                nc.sync.dma_start(
                    out=t_[:, j, :],
                    in_=xq[(2 * kp + j) * 128 : (2 * kp + j + 1) * 128, 0:QC],
                )
            xq0_t.append(t_)
        nc.sync.dma_start(out=wv_s, in_=wv[:])
        nc.sync.dma_start(out=wv_b, in_=wvb[:])
        nc.sync.dma_start(out=ones1, in_=cstc[0:1, :])
        for kt in range(KT_D):
            t_ = vstream.tile([128, SK], cdt, tag="xv", name="xv_s")
            nc.sync.dma_start(out=t_, in_=xv[kt * 128 : (kt + 1) * 128, :])
            xv_t.append(t_)
        nc.sync.dma_start(out=onesA[0:1, :], in_=cstc[1:2, :])
        nc.sync.dma_start(out=onesB[0:1, :], in_=cstc[2:3, :])
        for kp in range(KP):
            t_ = stream.tile([128, 2, S - QC], idt, tag="xqr", name="xqr_s")
            for j in range(2):
                nc.sync.dma_start(
                    out=t_[:, j, :],
                    in_=xq[(2 * kp + j) * 128 : (2 * kp + j + 1) * 128, QC:S],
                )
            xqr_t.append(t_)
        nc.sync.dma_start(out=wo_s, in_=wo[:])

        # ---------------- resident activations ----------------
        qT = [resid.tile([128, S], cdt, tag=f"qT{t}", name=f"qT{t}") for t in range(NT)]
        kT = [resid.tile([128, SK], cdt, tag=f"kT{t}", name=f"kT{t}") for t in range(NT)]
        v_s = resid.tile([128, ktk, VW], cdt, tag="v_s", name="v_s")
        cT = [resid.tile([128, S], cdt, tag=f"cT{t}", name=f"cT{t}") for t in range(NT)]

        def proj_mms(psum, w_s, t, src_of_kp, stop_w=None):
            # Q/K projection matmuls for one head-pair tile: fp8 DoubleRow
            # over kt-pairs (4 passes) or plain bf16 (8 passes)
            if KF8:
                for kp in range(KP):
                    nc.tensor.matmul(
                        psum,
                        lhsT=w_s[:, 2 * kp : 2 * kp + 2, t * 128 : (t + 1) * 128],
                        rhs=src_of_kp(kp),
                        start=(kp == 0),
                        stop=(kp == KP - 1),
                        perf_mode=MPD,
                    )
            else:
                for kp in range(KP):
                    for j in range(2):
                        nc.tensor.matmul(
                            psum,
                            lhsT=w_s[:, 2 * kp + j, t * 128 : (t + 1) * 128],
                            rhs=src_of_kp(kp)[:, j, :],
                            start=(kp == 0 and j == 0),
                            stop=(kp == KP - 1 and j == 1),
                        )

        # warmup operands: the HAM clock gate defaults to half rate and
        # needs ~3.4us of sustained PE activity to open; the first real
        # matmul is DMA-gated until ~13us, so run throwaway matmuls on a
        # memset tile first — K proj then starts at full clock
        wu = consts.tile([128, 512], BF16, tag="wu", name="wu")

        # ---------------- phase 1: K^T projection ----------------
        with tc.tile_pool(name="pp", bufs=1, space="PSUM") as pp:
            nc.vector.memset(wu[:], 1.0)
            psums = [
                pp.tile([128, QC], F32, tag=f"pp{i}", name=f"pp{i}")
                for i in range(NT * len(kchunks))
            ]
            for _ in range(10):
                nc.tensor.matmul(
                    psums[0][:], lhsT=wu[:, 0:128], rhs=wu[:], start=True,
                    stop=True,
                )
            for t in range(NT):
                for ci, (n0, w) in enumerate(kchunks):
                    proj_mms(
                        psums[t * len(kchunks) + ci][:, 0:w],
                        wk_s,
                        t,
                        lambda kp, n0=n0, w=w: xk_t[kp][:, :, n0 : n0 + w],
                    )
            for t in range(NT):
                for ci, (n0, w) in enumerate(kchunks):
                    nc.scalar.activation(
                        out=kT[t][:, n0 : n0 + w],
                        in_=psums[t * len(kchunks) + ci][:, 0:w],
                        func=AF.Identity,
                        bias=bk_s[:, t : t + 1],
                        scale=qsc,
                    )

        # ------- phases 2-4: V + Q projections woven into attention -------
        with tc.tile_pool(name="pa", bufs=1, space="PSUM") as pa:
            units = [(q0, w, p) for (q0, w) in QCHUNKS for p in range(NT)]

            def emit_vproj(m):
                # one V m-tile (all 4 heads + ones column); hooked into the
                # first unit's kt loop so it rides under the exp stream
                pvm = pa.tile([128, VW], F32, tag="ps", bufs=2, name="pv")
                for kt in range(KT_D):
                    nc.tensor.matmul(
                        pvm[:],
                        lhsT=xv_t[kt][:, m * 128 : (m + 1) * 128],
                        rhs=wv_s[:, kt, :],
                        start=(kt == 0),
                        stop=False,
                    )
                # bias + ones columns via augmented K=1 row
                nc.tensor.matmul(
                    pvm[:], lhsT=ones1[:], rhs=wv_b[:], start=False, stop=True
                )
                nc.vector.tensor_copy(v_s[:, m, :], pvm[:])

            def emit_qproj(qc, t, part=None):
                # one head-pair tile of one q-chunk of the Q projection,
                # split into two halves (part 0 / part 1) emitted in
                # CONSECUTIVE inject slots, so each PE convoy stays under
                # ~1us and the ACT exp stream never drains; the qp psum
                # tile is carried across in `qparts`
                qsl = slice(qc * QC, (qc + 1) * QC)
                if part in (None, 0):
                    qp = pa.tile([128, QC], F32, tag="ps", bufs=2, name="qp")
                    qparts[(qc, t)] = qp
                else:
                    qp = qparts.pop((qc, t))

                def src(kp):
                    if qc == 0:
                        return xq0_t[kp][:, :, :]
                    sl = slice((qc - 1) * QC, qc * QC)
                    return xqr_t[kp][:, :, sl]

                kps = range(KP) if part is None else (
                    range(KP // 2) if part == 0 else range(KP // 2, KP)
                )
                if KF8:
                    for kp in kps:
                        nc.tensor.matmul(
                            qp[:],
                            lhsT=wq_s[:, 2 * kp : 2 * kp + 2, t * 128 : (t + 1) * 128],
                            rhs=src(kp),
                            start=(kp == 0),
                            stop=(kp == KP - 1),
                            perf_mode=MPD,
                        )
                else:
                    for kp in kps:
                        for j in range(2):
                            nc.tensor.matmul(
                                qp[:],
                                lhsT=wq_s[:, 2 * kp + j, t * 128 : (t + 1) * 128],
                                rhs=src(kp)[:, j, :],
                                start=(kp == 0 and j == 0),
                                stop=(kp == KP - 1 and j == 1),
                            )
                if part == 0:
                    return
                # DVE evac (unscale + bias add + cast): ACT's exp backlog
                # would hold the qp psum slot hostage for several us
                if KF8:
                    nc.vector.tensor_scalar(
                        out=qT[t][:, qsl],
                        in0=qp[:],
                        scalar1=qsc,
                        scalar2=bq_s[:, t : t + 1],
                        op0=ALU.mult,
                        op1=ALU.add,
                    )
                else:
                    nc.vector.tensor_scalar_add(
                        qT[t][:, qsl], qp[:], bq_s[:, t : t + 1]
                    )

            def emit_score(st, s):
                # one kt-slot of score matmuls + exp for a unit
                q0, w, p = st["unit"]
                qsl = slice(q0, q0 + w)
                ksl = slice(s * 128, (s + 1) * 128)
                # per-head row stride padded to QC so both heads' matmul
                # destinations stay PSUM-bank-aligned even for the narrow
                # tail units
                ps = pa.tile([128, 2, QC], F32, tag="ps", bufs=2, name="ps")
                nc.tensor.matmul(
                    ps[:, 0, 0:w],
                    lhsT=kT[p][0:64, ksl],
                    rhs=qT[p][0:64, qsl],
                    start=True,
                    stop=True,
                )
                nc.tensor.matmul(
                    ps[:, 1, 0:w],
                    lhsT=kT[p][64:128, ksl],
                    rhs=qT[p][64:128, qsl],
                    start=True,
                    stop=True,
                )
                pt = ptp.tile([128, 2, w], cdt, tag="pt", name="pt")
                nc.scalar.activation(
                    out=pt[:],
                    in_=ps[:, :, 0:w],
                    func=AF.Exp,
                    bias=mb_s[:, s : s + 1],
                    scale=SCALE,
                )
                st["pts"].append(pt)

            def emit_ctx(st, ct):
                q0, w, p = st["unit"]
                hA, hB = 2 * p, 2 * p + 1
                ptc = st["pts"][ct]
                nc.tensor.matmul(
                    st["pcA"][0:65, 0:w],
                    lhsT=v_s[:, ct, hA * 65 : (hA + 1) * 65],
                    rhs=ptc[:, 0, :],
                    start=(ct == 0),
                    stop=(ct == ktk - 1),
                )
                nc.tensor.matmul(
                    st["pcB"][0:65, 0:w],
                    lhsT=v_s[:, ct, hB * 65 : (hB + 1) * 65],
                    rhs=ptc[:, 1, :],
                    start=(ct == 0),
                    stop=(ct == ktk - 1),
                )

            def emit_recips(st):
                # fast-approx reciprocal (custom DVE, ~18 bits; must read
                # SBUF — PSUM-source custom-DVE misbehaves on hw) on the
                # denominator rows, then one cast to the compute dtype for
                # the PE broadcast
                _, w, _ = st["unit"]
                pcA, pcB = st["pcA"], st["pcB"]
                den = smalls.tile([1, 2 * QC], F32, tag="den", name="den")
                rec32 = smalls.tile([1, 2 * QC], F32, tag="rec32", name="rec32")
                rec = smalls.tile([1, 2 * QC], cdt, tag="rec", name="rec")
                nc.vector.tensor_copy(den[0:1, 0:w], pcA[64:65, 0:w])
                nc.vector.tensor_copy(den[0:1, w : 2 * w], pcB[64:65, 0:w])
                nc.vector.reciprocal_approx_fast(
                    out=rec32[0:1, 0 : 2 * w], in_=den[0:1, 0 : 2 * w]
                )
                nc.vector.tensor_copy(rec[0:1, 0 : 2 * w], rec32[0:1, 0 : 2 * w])
                st["rec"] = rec

            def emit_norm(st):
                # partition-broadcast of the reciprocal row via two K=1
                # outer-product matmuls, then DVE multiplies
                q0, w, p = st["unit"]
                qsl = slice(q0, q0 + w)
                pcA, pcB, rec = st["pcA"], st["pcB"], st["rec"]
                pbc = pa.tile([128, w], F32, tag="ps", bufs=2, name="pbc")
                nc.tensor.matmul(
                    pbc[:],
                    lhsT=onesA[0:1, :],
                    rhs=rec[0:1, 0:w],
                    start=True,
                    stop=False,
                )
                nc.tensor.matmul(
                    pbc[:],
                    lhsT=onesB[0:1, :],
                    rhs=rec[0:1, w : 2 * w],
                    start=False,
                    stop=True,
                )
                bcs = smalls.tile([128, QC], F32, tag="bcs", name="bcs")
                nc.vector.tensor_copy(bcs[:, 0:w], pbc[:])
                nc.vector.tensor_mul(
                    cT[p][0:64, qsl], pcA[0:64, 0:w], bcs[0:64, 0:w]
                )
                nc.vector.tensor_mul(
                    cT[p][64:128, qsl], pcB[0:64, 0:w], bcs[64:128, 0:w]
                )

            def emit_final(ms, part=None):
                # O-projection m-tiles; with part 0/1 the two t-halves are
                # emitted in CONSECUTIVE inject slots (~0.9us PE convoys)
                for m in ms:
                    if part in (None, 0):
                        pom = pa.tile([128, D], F32, tag="ps", bufs=2, name="pom")
                        oparts[m] = pom
                    else:
                        pom = oparts.pop(m)
                    ts = range(NT) if part is None else [part]
                    for t in ts:
                        for oc in range(2):
                            nc.tensor.matmul(
                                pom[:, oc * 512 : (oc + 1) * 512],
                                lhsT=cT[t][:, m * 128 : (m + 1) * 128],
                                rhs=wo_s[:, t, oc * 512 : (oc + 1) * 512],
                                start=(t == 0),
                                stop=(t == NT - 1),
                            )
                    if part == 0:
                        continue
                    ob = obp.tile([128, D], odt, tag="ob", name="ob")
                    # DVE copy (gpsimd can't read PSUM): ACT is saturated by
                    # the exp() stream
                    nc.vector.tensor_copy(ob[:], pom[:])
                    nc.sync.dma_start(out=out[m * 128 : (m + 1) * 128, :], in_=ob[:])

            # --------------- per-unit emission with injection ---------------
            # O-projection m-tile schedule: eligibility-driven spread;
            # m-tiles of q-chunk c are ready at iteration 2c+3 (both its
            # units norm'd), leaving only m15 after the loop.
            OSCHED = {3: [0, 1], 4: [2, 3], 5: [4, 5], 6: [6, 7],
                      7: [8, 9], 8: [10, 11], 9: [12, 13, 14]}

            # Unit 0 runs with a deep ctx lag: its exp stream starts as soon
            # as xq chunk 0 + kT are ready (~8us before xv finishes), and
            # the V projection tiles are hooked in just after xv lands, each
            # one kt slot ahead of the ctx matmul that consumes it.
            LAG0 = min(4, ktk)

            states = []
            qparts = {}
            oparts = {}
            emit_qproj(0, 0)
            emit_qproj(0, 1)
            for i, (q0, w, p) in enumerate(units):
                # Auxiliary PE work (previous unit's normalization broadcast,
                # O-proj m-tiles, next q-chunk projection halves) is injected
                # one item per kt slot INSIDE the attention stream, so the
                # score matmuls — and with them the ACT exp stream, the
                # kernel's pacing engine — never see a multi-us convoy of
                # other PE work.
                lag = LAG0 if i == 0 else 2
                st = {"unit": (q0, w, p), "pts": [],
                      "pcA": pa.tile([65, QC], F32, tag="pcA", bufs=2, name="pcA"),
                      "pcB": pa.tile([65, QC], F32, tag="pcB", bufs=2, name="pcB"),
                      "rec": None}
                if i == 0:
                    work = [
                        (lambda m=m: emit_vproj(m)) for m in range(ktk)
                    ][::-1]
                else:
                    items = [lambda j=i - 1: emit_norm(states[j])]
                    for m in OSCHED.get(i, []):
                        items.append(lambda m=m: emit_final([m], 0))
                        items.append(lambda m=m: emit_final([m], 1))
                    if i in (1, 3, 5):
                        qc = i // 2 + 1
                        for t in range(NT):
                            items.append(lambda qc=qc, t=t: emit_qproj(qc, t, 0))
                            items.append(lambda qc=qc, t=t: emit_qproj(qc, t, 1))
                    work = list(reversed(items))
                states.append(st)

                ioff = LAG0 - 1 if i == 0 else 2
                for kt in range(ktk + lag):
                    if kt < ktk:
                        emit_score(st, kt)
                    if kt >= ioff and work and kt < ktk + lag - 1:
                        work.pop()()
                    if kt >= lag:
                        emit_ctx(st, kt - lag)
                while work:
                    work.pop()()
                emit_recips(st)
            emit_norm(states[-1])
            emit_final([15])

    nc.compile()
    return nc


def _const_rows():
    cst = np.zeros((3, 128), np.float32)
    cst[0, :] = 1.0
    cst[1, 0:64] = 1.0
    cst[2, 64:128] = 1.0
    return cst


def make_in_maps(query, key, value, mask, Wq, bq, Wk, bk, Wv, bv, Wo, bo):
    """Returns (in_maps, ktk). Key positions with mask=True are dropped."""
    query = np.asarray(query, np.float32)
    key = np.asarray(key, np.float32)
    value = np.asarray(value, np.float32)
    mask = np.asarray(mask)
    Wq = np.asarray(Wq, np.float32)
    Wk = np.asarray(Wk, np.float32)
    Wv = np.asarray(Wv, np.float32)
    Wo = np.asarray(Wo, np.float32)
    bq = np.asarray(bq, np.float32)
    bk = np.asarray(bk, np.float32)
    bv = np.asarray(bv, np.float32)

    keep = [np.flatnonzero(~mask[b, 0]) for b in range(B)]
    ktk = max(1, max((len(k) + 127) // 128 for k in keep))
    SKc = 128 * ktk
    ndt = _np_dt()
    if KF8:
        idt = mybir.dt.np(FP8)

        def _q8(a):
            return np.clip(a, -240.0, 240.0).astype(idt)
    else:
        idt = ndt
        _q8 = None

    def _prearrange(w):
        # [D, n] -> [128, KT_D, n] matching the SBUF tile layout
        n = w.shape[1]
        return np.ascontiguousarray(
            w.reshape(KT_D, 128, n).transpose(1, 0, 2)
        )

    in_maps = []
    for c in range(NCORES):
        b, g = c // G, c % G
        cs = slice(g * DG, (g + 1) * DG)
        idx = keep[b]
        nk = len(idx)
        xkc = np.zeros((D, SKc), np.float32)
        xvc = np.zeros((D, SKc), np.float32)
        xkc[:, :nk] = key[b].T[:, idx]
        xvc[:, :nk] = value[b].T[:, idx]
        mbias = np.full(SKc, MASK_NEG, np.float32)
        mbias[:nk] = 0.0

        wv_aug = np.zeros((D, VW), np.float32)
        wvb_row = np.zeros((1, VW), np.float32)
        for j in range(HPG):
            src = slice(g * DG + j * DK, g * DG + (j + 1) * DK)
            wv_aug[:, j * 65 : j * 65 + 64] = Wv[:, src]
            wvb_row[0, j * 65 : j * 65 + 64] = bv[src]
            wvb_row[0, j * 65 + 64] = 1.0

        # wo: [DG, D] -> [128, NT, D]
        wo_pre = np.ascontiguousarray(
            Wo[cs, :].reshape(NT, 128, D).transpose(1, 0, 2)
        )

        xq_c = np.ascontiguousarray(query[b].T)
        wq_c = _prearrange(Wq[:, cs])
        wk_c = _prearrange(Wk[:, cs])
        if KF8:
            xq_m, xk_m = _q8(xq_c), _q8(xkc)
            wq_m, wk_m = _q8(wq_c * WSC), _q8(wk_c * WSC)
        else:
            xq_m, xk_m = xq_c.astype(ndt), xkc.astype(ndt)
            wq_m, wk_m = wq_c.astype(ndt), wk_c.astype(ndt)

        in_maps.append(
            {
                "xq": xq_m,
                "xk": xk_m,
                "xv": xvc.astype(ndt),
                "wq": wq_m,
                "wk": wk_m,
                "wv": _prearrange(wv_aug).astype(ndt),
                "wvb": wvb_row.astype(ndt),
                "wo": wo_pre.astype(ndt),
                "bq": np.ascontiguousarray(bq[cs].reshape(NT, 128).T),
                "bk": np.ascontiguousarray(bk[cs].reshape(NT, 128).T),
                "mb": np.ascontiguousarray(mbias.reshape(ktk, 128).T),
                "cstc": _const_rows().astype(ndt),
            }
        )
    return in_maps, ktk


def combine_outputs(results, mask, bo):
    mask = np.asarray(mask)
    bo = np.asarray(bo, np.float32)
    out = np.zeros((B, S, D), np.float32)
    for c in range(NCORES):
        out[c // G] += np.asarray(results[c]["out"], np.float32)
    for b in range(B):
        if mask[b, 0].all():
            # reference: fully-masked rows produce zero context
            out[b] = 0.0
    out += bo[None, None, :]
    return out


_NC_CACHE = {}


def kernel(query, key, value, mask, Wq, bq, Wk, bk, Wv, bv, Wo, bo):
    from concourse.bass_utils import run_bass_kernel_spmd

    in_maps, ktk = make_in_maps(
        query, key, value, mask, Wq, bq, Wk, bk, Wv, bv, Wo, bo
    )
    nc = _NC_CACHE.get((KDT, KF8, ktk))
    if nc is None:
        nc = _NC_CACHE[(KDT, KF8, ktk)] = build_bass(ktk)
    res = run_bass_kernel_spmd(nc, in_maps, list(range(NCORES))).results
    return combine_outputs(res, mask, bo)


# revision 48
# speedup vs baseline: 1.1995x; 1.1995x over previous
"""Multi-head attention (B=2, S=2048, D=1024, H=16) on 8 NeuronCores.

Sharding: core c -> (batch b = c // 4, head-group g = c % 4). Each core
computes 4 heads of one batch plus the partial output projection for its
head-group's rows of Wo; the host sums the 4 partials per batch and adds bo.

Key-side compaction: masked key positions (True in `mask`) contribute
exactly zero attention weight, so the host drops them before sharding —
key/value inputs, K/V projections, score matmuls, the exp() pass and the
ctx matmuls all shrink by the masked fraction. The compacted length is
padded to a multiple of 128 with zero-columns whose mask bias (-60, applied
inside the exp activation) keeps their contribution at ~1e-26.

Layout strategy (per core):
  - Inputs are host-transposed: x^T [D, S*] so projections run with W as the
    stationary operand and x^T as the moving operand. Weights are
    host-prearranged into the [128, kt, n] SBUF layout so their DMA is one
    contiguous descriptor per partition row.
  - Q/K projections run in fp8-e4m3 with DoubleRow perf mode: x and W are
    quantized host-side (W scaled by 16 to stay in e4m3 normal range; the
    1/16 folds into the PSUM evacuation), and each matmul contracts TWO
    128-row D-tiles at once, halving the projection pass count. Errors of
    the fp8 quantization average out in the length-1024 dot products
    (~0.2% on q/k, ~0.6% on logits). Q^T/K^T themselves stay bf16.
  - Scores are computed TRANSPOSED: S^T[k, q] = K Q^T, so the key-position
    (padding) mask is per-PARTITION and folds into the single exp()
    activation as a bias AP, along with the 1/sqrt(dk) scale. One exp per
    [128, 2, w] PSUM tile covers both heads of a pair (the two heads' score
    matmuls run concurrently via PE row-tiling, K=64 each; the per-head row
    stride is padded to 512 so both destinations stay PSUM-bank-aligned).
  - V is produced in natural [S*, dv] layout with a ones-column per head
    (bias folded via an augmented contraction row), so the ctx matmul
    ctx^T = [V_h | 1]^T @ P^T also yields the softmax denominator as row 64.
  - Normalization: the denominator rows are copied to SBUF, reciprocal'd
    with the custom-DVE fast approximation (~18-bit, one instruction), cast
    to bf16, broadcast across partitions with two K=1 outer-product matmuls,
    then DVE multiplies. Pipelined one unit behind the matmul blocks.
  - Pipeline shape: K proj first, then 10 attention units (chunk, head-pair)
    with scores/exp running `lag` kt-slots ahead of ctx. ALL auxiliary PE
    work — V-proj m-tiles (unit 0), the previous unit's norm broadcast,
    O-proj m-tiles, Q-proj pair-tiles — is injected one item per kt slot
    INSIDE the attention stream, so the score matmuls (and the ACT exp
    stream they feed) never see a multi-us convoy of other PE work and the
    PE stays dense enough to hold the HAM clock gate at full rate.
  - Tail: the last q-chunk is split 384+128 and O-projection m-tiles are
    spread on an eligibility-driven schedule, so after the final unit's
    normalization only one 128-row m-tile remains.

Compute dtype (env KDT): "bf16" (default) uses bfloat16 matmul operands
(~5e-3 rel err, 1 cyc/row PE + half the DMA of f32); "f32r" keeps float32r
operands (~2e-4 rel err but ~2x slower matmuls). KF8=0 disables the fp8
projection path.
"""

import os
from contextlib import ExitStack

import numpy as np

import concourse.bacc as bacc
import concourse.mybir as mybir
import concourse.tile as tile

F32 = mybir.dt.float32
F32R = mybir.dt.float32r
BF16 = mybir.dt.bfloat16
FP8 = mybir.dt.float8e4
AF = mybir.ActivationFunctionType
ALU = mybir.AluOpType
MPD = mybir.MatmulPerfMode.DoubleRow

B, S, D = 2, 2048, 1024
H, DK = 16, 64
G = 4                    # head-groups (tensor parallel)
HPG = H // G             # 4 heads per group
DG = HPG * DK            # 256 head dims per group
NCORES = 8
MASK_NEG = -60.0         # additive post-scale bias for padded key positions
SCALE = 0.125            # 1/sqrt(dk)
WSC = 16.0               # host-side W_q/W_k scale for fp8 normal range

KT_D = D // 128          # 8 contraction tiles for projections
KP = KT_D // 2           # 4 DoubleRow kt-pairs
NT = DG // 128           # 2 partition-tiles of qT/kT/cT (one head-pair each)
QC = 512                 # q projection chunk (matmul moving dim)
NQC = S // QC            # 4
# attention-unit q chunks: last 512 chunk split 384+128 so the tail after
# the final normalization is a single O-proj m-tile
QCHUNKS = [(0, 512), (512, 512), (1024, 512), (1536, 384), (1920, 128)]
VW = HPG * (DK + 1)      # 260: V width incl. per-head ones column

KDT = os.environ.get("KDT", "bf16")
KF8 = os.environ.get("KF8", "1") == "1" and KDT == "bf16"


def _dt():
    return BF16 if KDT == "bf16" else F32R


def _np_dt():
    import ml_dtypes

    return ml_dtypes.bfloat16 if KDT == "bf16" else np.float32


def build_bass(ktk):
    """Build the SPMD program for `ktk` 128-wide key tiles (SK = 128*ktk)."""
    SK = 128 * ktk
    kchunks = [(n0, min(QC, SK - n0)) for n0 in range(0, SK, QC)]
    cdt = _dt()
    idt = FP8 if KF8 else cdt      # x_q/x_k + W_q/W_k dtype
    qsc = 1.0 / WSC if KF8 else 1.0

    nc = bacc.Bacc(None, target_bir_lowering=False, debug=False)

    xq = nc.dram_tensor("xq", [D, S], idt, kind="ExternalInput")
    xk = nc.dram_tensor("xk", [D, SK], idt, kind="ExternalInput")
    xv = nc.dram_tensor("xv", [D, SK], cdt, kind="ExternalInput")
    # weights pre-arranged host-side into the SBUF tile layout (contiguous
    # per-partition rows -> few large DMA descriptors)
    wq = nc.dram_tensor("wq", [128, KT_D, DG], idt, kind="ExternalInput")
    wk = nc.dram_tensor("wk", [128, KT_D, DG], idt, kind="ExternalInput")
    wv = nc.dram_tensor("wv", [128, KT_D, VW], cdt, kind="ExternalInput")
    wvb = nc.dram_tensor("wvb", [1, VW], cdt, kind="ExternalInput")
    wo = nc.dram_tensor("wo", [128, NT, D], cdt, kind="ExternalInput")
    bq = nc.dram_tensor("bq", [128, NT], F32, kind="ExternalInput")
    bk = nc.dram_tensor("bk", [128, NT], F32, kind="ExternalInput")
    mb = nc.dram_tensor("mb", [128, ktk], F32, kind="ExternalInput")
    cstc = nc.dram_tensor("cstc", [3, 128], cdt, kind="ExternalInput")
    # bf16 partials: the host sums 4 head-group partials per batch in f32,
    # so the extra rounding is ~0.1% while output DMA bytes halve
    odt = BF16 if KDT == "bf16" else F32
    out = nc.dram_tensor("out", [S, D], odt, kind="ExternalOutput")

    with tile.TileContext(nc) as tc, ExitStack() as ctx:
        consts = ctx.enter_context(tc.tile_pool(name="consts", bufs=1))
        resid = ctx.enter_context(tc.tile_pool(name="resid", bufs=1))
        stream = ctx.enter_context(tc.tile_pool(name="stream", bufs=4))
        vstream = ctx.enter_context(tc.tile_pool(name="vstream", bufs=8))
        ptp = ctx.enter_context(tc.tile_pool(name="ptp", bufs=10 if ktk <= 12 else 4))
        smalls = ctx.enter_context(tc.tile_pool(name="smalls", bufs=3 if ktk <= 12 else 2))
        obp = ctx.enter_context(tc.tile_pool(name="obp", bufs=3))

        # ---------------- constants / weights declarations ----------------
        wq_s = consts.tile([128, KT_D, DG], idt, tag="wq_s", name="wq_s")
        bq_s = consts.tile([128, NT], F32, tag="bq_s", name="bq_s")
        wk_s = consts.tile([128, KT_D, DG], idt, tag="wk_s", name="wk_s")
        bk_s = consts.tile([128, NT], F32, tag="bk_s", name="bk_s")
        wv_s = consts.tile([128, KT_D, VW], cdt, tag="wv_s", name="wv_s")
        wv_b = consts.tile([1, VW], cdt, tag="wv_b", name="wv_b")
        wo_s = consts.tile([128, NT, D], cdt, tag="wo_s", name="wo_s")
        mb_s = consts.tile([128, ktk], F32, tag="mb_s", name="mb_s")
        # Constant rows (all-ones, head-A selector, head-B selector) come
        # from tiny DRAM inputs — memset can't write float32r tiles.
        ones1 = consts.tile([1, 128], cdt, tag="ones1", name="ones1")
        onesA = consts.tile([1, 128], cdt, tag="onesA", name="onesA")
        onesB = consts.tile([1, 128], cdt, tag="onesB", name="onesB")

        # ---------------- input stream prefetch ----------------
        # DMA issue order sets time-to-first-exp (the ACT exp stream is the
        # kernel's long pole): wk+xk first (K proj is the first PE work),
        # then wq + the q-chunk-0 slice of xq (first attention unit), xv
        # (V must finish right before the first ctx matmuls), then the rest
        # of xq streaming in under the attention phase. The x streams land
        # in kt-PAIR tiles [128, 2, w] so the fp8 DoubleRow matmuls can
        # address both contraction tiles in one AP.
        nc.sync.dma_start(out=wk_s, in_=wk[:])
        nc.sync.dma_start(out=bk_s, in_=bk[:])
        nc.sync.dma_start(out=mb_s, in_=mb[:])
        xk_t, xv_t, xq0_t, xqr_t = [], [], [], []
        for kp in range(KP):
            t_ = stream.tile([128, 2, SK], idt, tag="xk", name="xk_s")
            for j in range(2):
                nc.sync.dma_start(
                    out=t_[:, j, :],
                    in_=xk[(2 * kp + j) * 128 : (2 * kp + j + 1) * 128, :],
                )
            xk_t.append(t_)
        nc.sync.dma_start(out=wq_s, in_=wq[:])
        nc.sync.dma_start(out=bq_s, in_=bq[:])
        for kp in range(KP):
            t_ = stream.tile([128, 2, QC], idt, tag="xq0", name="xq0_s")
            for j in range(2):
                nc.sync.dma_start(
                    out=t_[:, j, :],
                    in_=xq[(2 * kp + j) * 128 : (2 * kp + j + 1) * 128, 0:QC],
                )
            xq0_t.append(t_)
        nc.sync.dma_start(out=wv_s, in_=wv[:])
        nc.sync.dma_start(out=wv_b, in_=wvb[:])
        nc.sync.dma_start(out=ones1, in_=cstc[0:1, :])
        for kt in range(KT_D):
            t_ = vstream.tile([128, SK], cdt, tag="xv", name="xv_s")
            nc.sync.dma_start(out=t_, in_=xv[kt * 128 : (kt + 1) * 128, :])
            xv_t.append(t_)
        nc.sync.dma_start(out=onesA[0:1, :], in_=cstc[1:2, :])
        nc.sync.dma_start(out=onesB[0:1, :], in_=cstc[2:3, :])
        for kp in range(KP):
            t_ = stream.tile([128, 2, S - QC], idt, tag="xqr", name="xqr_s")
            for j in range(2):
                nc.sync.dma_start(
                    out=t_[:, j, :],
                    in_=xq[(2 * kp + j) * 128 : (2 * kp + j + 1) * 128, QC:S],
                )
            xqr_t.append(t_)
        nc.sync.dma_start(out=wo_s, in_=wo[:])

        # ---------------- resident activations ----------------
        qT = [resid.tile([128, S], cdt, tag=f"qT{t}", name=f"qT{t}") for t in range(NT)]
        kT = [resid.tile([128, SK], cdt, tag=f"kT{t}", name=f"kT{t}") for t in range(NT)]
        v_s = resid.tile([128, ktk, VW], cdt, tag="v_s", name="v_s")
        cT = [resid.tile([128, S], cdt, tag=f"cT{t}", name=f"cT{t}") for t in range(NT)]

        def proj_mms(psum, w_s, t, src_of_kp, stop_w=None):
            # Q/K projection matmuls for one head-pair tile: fp8 DoubleRow
            # over kt-pairs (4 passes) or plain bf16 (8 passes)
            if KF8:
                for kp in range(KP):
                    nc.tensor.matmul(
                        psum,
                        lhsT=w_s[:, 2 * kp : 2 * kp + 2, t * 128 : (t + 1) * 128],
                        rhs=src_of_kp(kp),
                        start=(kp == 0),
                        stop=(kp == KP - 1),
                        perf_mode=MPD,
                    )
            else:
                for kp in range(KP):
                    for j in range(2):
                        nc.tensor.matmul(
                            psum,
                            lhsT=w_s[:, 2 * kp + j, t * 128 : (t + 1) * 128],
                            rhs=src_of_kp(kp)[:, j, :],
                            start=(kp == 0 and j == 0),
                            stop=(kp == KP - 1 and j == 1),
                        )

        # warmup operands: the HAM clock gate defaults to half rate and
        # needs ~3.4us of sustained PE activity to open; the first real
        # matmul is DMA-gated until ~13us, so run throwaway matmuls on a
        # memset tile first — K proj then starts at full clock
        wu = consts.tile([128, 512], BF16, tag="wu", name="wu")

        # ---------------- phase 1: K^T projection ----------------
        with tc.tile_pool(name="pp", bufs=1, space="PSUM") as pp:
            nc.vector.memset(wu[:], 1.0)
            psums = [
                pp.tile([128, QC], F32, tag=f"pp{i}", name=f"pp{i}")
                for i in range(NT * len(kchunks))
            ]
            for _ in range(10):
                nc.tensor.matmul(
                    psums[0][:], lhsT=wu[:, 0:128], rhs=wu[:], start=True,
                    stop=True,
                )
            for t in range(NT):
                for ci, (n0, w) in enumerate(kchunks):
                    proj_mms(
                        psums[t * len(kchunks) + ci][:, 0:w],
                        wk_s,
                        t,
                        lambda kp, n0=n0, w=w: xk_t[kp][:, :, n0 : n0 + w],
                    )
            for t in range(NT):
                for ci, (n0, w) in enumerate(kchunks):
                    nc.scalar.activation(
                        out=kT[t][:, n0 : n0 + w],
                        in_=psums[t * len(kchunks) + ci][:, 0:w],
                        func=AF.Identity,
                        bias=bk_s[:, t : t + 1],
                        scale=qsc,
                    )

        # ------- phases 2-4: V + Q projections woven into attention -------
        with tc.tile_pool(name="pa", bufs=1, space="PSUM") as pa:
            units = [(q0, w, p) for (q0, w) in QCHUNKS for p in range(NT)]

            def emit_vproj(m):
                # one V m-tile (all 4 heads + ones column); hooked into the
                # first unit's kt loop so it rides under the exp stream
                pvm = pa.tile([128, VW], F32, tag="pj", bufs=2, name="pv")
                for kt in range(KT_D):
                    nc.tensor.matmul(
                        pvm[:],
                        lhsT=xv_t[kt][:, m * 128 : (m + 1) * 128],
                        rhs=wv_s[:, kt, :],
                        start=(kt == 0),
                        stop=False,
                    )
                # bias + ones columns via augmented K=1 row
                nc.tensor.matmul(
                    pvm[:], lhsT=ones1[:], rhs=wv_b[:], start=False, stop=True
                )
                nc.vector.tensor_copy(v_s[:, m, :], pvm[:])

            def emit_qproj(qc, t, part=None):
                # one head-pair tile of one q-chunk of the Q projection,
                # split into two halves (part 0 / part 1) emitted in
                # CONSECUTIVE inject slots, so each PE convoy stays under
                # ~1us and the ACT exp stream never drains; the qp psum
                # tile is carried across in `qparts`
                qsl = slice(qc * QC, (qc + 1) * QC)
                if part in (None, 0):
                    qp = pa.tile([128, QC], F32, tag="pj", bufs=2, name="qp")
                    qparts[(qc, t)] = qp
                else:
                    qp = qparts.pop((qc, t))

                def src(kp):
                    if qc == 0:
                        return xq0_t[kp][:, :, :]
                    sl = slice((qc - 1) * QC, qc * QC)
                    return xqr_t[kp][:, :, sl]

                kps = range(KP) if part is None else (
                    range(KP // 2) if part == 0 else range(KP // 2, KP)
                )
                if KF8:
                    for kp in kps:
                        nc.tensor.matmul(
                            qp[:],
                            lhsT=wq_s[:, 2 * kp : 2 * kp + 2, t * 128 : (t + 1) * 128],
                            rhs=src(kp),
                            start=(kp == 0),
                            stop=(kp == KP - 1),
                            perf_mode=MPD,
                        )
                else:
                    for kp in kps:
                        for j in range(2):
                            nc.tensor.matmul(
                                qp[:],
                                lhsT=wq_s[:, 2 * kp + j, t * 128 : (t + 1) * 128],
                                rhs=src(kp)[:, j, :],
                                start=(kp == 0 and j == 0),
                                stop=(kp == KP - 1 and j == 1),
                            )
                if part == 0:
                    return
                # DVE evac (unscale + bias add + cast): ACT's exp backlog
                # would hold the qp psum slot hostage for several us
                if KF8:
                    nc.vector.tensor_scalar(
                        out=qT[t][:, qsl],
                        in0=qp[:],
                        scalar1=qsc,
                        scalar2=bq_s[:, t : t + 1],
                        op0=ALU.mult,
                        op1=ALU.add,
                    )
                else:
                    nc.vector.tensor_scalar_add(
                        qT[t][:, qsl], qp[:], bq_s[:, t : t + 1]
                    )

            def emit_score(st, s):
                # one kt-slot of score matmuls + exp for a unit
                q0, w, p = st["unit"]
                qsl = slice(q0, q0 + w)
                ksl = slice(s * 128, (s + 1) * 128)
                # per-head row stride padded to QC so both heads' matmul
                # destinations stay PSUM-bank-aligned even for the narrow
                # tail units
                ps = pa.tile([128, 2, QC], F32, tag="ps", bufs=2, name="ps")
                nc.tensor.matmul(
                    ps[:, 0, 0:w],
                    lhsT=kT[p][0:64, ksl],
                    rhs=qT[p][0:64, qsl],
                    start=True,
                    stop=True,
                )
                nc.tensor.matmul(
                    ps[:, 1, 0:w],
                    lhsT=kT[p][64:128, ksl],
                    rhs=qT[p][64:128, qsl],
                    start=True,
                    stop=True,
                )
                pt = ptp.tile([128, 2, w], cdt, tag="pt", name="pt")
                nc.scalar.activation(
                    out=pt[:],
                    in_=ps[:, :, 0:w],
                    func=AF.Exp,
                    bias=mb_s[:, s : s + 1],
                    scale=SCALE,
                )
                st["pts"].append(pt)

            def emit_ctx(st, ct):
                q0, w, p = st["unit"]
                hA, hB = 2 * p, 2 * p + 1
                ptc = st["pts"][ct]
                nc.tensor.matmul(
                    st["pcA"][0:65, 0:w],
                    lhsT=v_s[:, ct, hA * 65 : (hA + 1) * 65],
                    rhs=ptc[:, 0, :],
                    start=(ct == 0),
                    stop=(ct == ktk - 1),
                )
                nc.tensor.matmul(
                    st["pcB"][0:65, 0:w],
                    lhsT=v_s[:, ct, hB * 65 : (hB + 1) * 65],
                    rhs=ptc[:, 1, :],
                    start=(ct == 0),
                    stop=(ct == ktk - 1),
                )

            def emit_recips(st):
                # fast-approx reciprocal (custom DVE, ~18 bits; must read
                # SBUF — PSUM-source custom-DVE misbehaves on hw) on the
                # denominator rows, then one cast to the compute dtype for
                # the PE broadcast
                _, w, _ = st["unit"]
                pcA, pcB = st["pcA"], st["pcB"]
                den = smalls.tile([1, 2 * QC], F32, tag="den", name="den")
                rec32 = smalls.tile([1, 2 * QC], F32, tag="rec32", name="rec32")
                rec = smalls.tile([1, 2 * QC], cdt, tag="rec", name="rec")
                nc.vector.tensor_copy(den[0:1, 0:w], pcA[64:65, 0:w])
                nc.vector.tensor_copy(den[0:1, w : 2 * w], pcB[64:65, 0:w])
                nc.vector.reciprocal_approx_fast(
                    out=rec32[0:1, 0 : 2 * w], in_=den[0:1, 0 : 2 * w]
                )
                nc.vector.tensor_copy(rec[0:1, 0 : 2 * w], rec32[0:1, 0 : 2 * w])
                st["rec"] = rec

            def emit_norm(st):
                # partition-broadcast of the reciprocal row via two K=1
                # outer-product matmuls, then DVE multiplies
                q0, w, p = st["unit"]
                qsl = slice(q0, q0 + w)
                pcA, pcB, rec = st["pcA"], st["pcB"], st["rec"]
                pbc = pa.tile([128, w], F32, tag="pj", bufs=2, name="pbc")
                nc.tensor.matmul(
                    pbc[:],
                    lhsT=onesA[0:1, :],
                    rhs=rec[0:1, 0:w],
                    start=True,
                    stop=False,
                )
                nc.tensor.matmul(
                    pbc[:],
                    lhsT=onesB[0:1, :],
                    rhs=rec[0:1, w : 2 * w],
                    start=False,
                    stop=True,
                )
                bcs = smalls.tile([128, QC], F32, tag="bcs", name="bcs")
                nc.vector.tensor_copy(bcs[:, 0:w], pbc[:])
                nc.vector.tensor_mul(
                    cT[p][0:64, qsl], pcA[0:64, 0:w], bcs[0:64, 0:w]
                )
                nc.vector.tensor_mul(
                    cT[p][64:128, qsl], pcB[0:64, 0:w], bcs[64:128, 0:w]
                )

            def emit_final(ms, part=None):
                # O-projection m-tiles; with part 0/1 the two t-halves are
                # emitted in CONSECUTIVE inject slots (~0.9us PE convoys)
                for m in ms:
                    ob = obp.tile([128, D], odt, tag="ob", name="ob")
                    for oc in range(2):
                        pom = pa.tile([128, 512], F32, tag="pj", bufs=2, name="pom")
                        for t in range(NT):
                            nc.tensor.matmul(
                                pom[:],
                                lhsT=cT[t][:, m * 128 : (m + 1) * 128],
                                rhs=wo_s[:, t, oc * 512 : (oc + 1) * 512],
                                start=(t == 0),
                                stop=(t == NT - 1),
                            )
                        # DVE copy (gpsimd can't read PSUM): ACT is
                        # saturated by the exp() stream
                        nc.vector.tensor_copy(
                            ob[:, oc * 512 : (oc + 1) * 512], pom[:]
                        )
                    nc.sync.dma_start(out=out[m * 128 : (m + 1) * 128, :], in_=ob[:])

            # --------------- per-unit emission with injection ---------------
            # O-projection m-tile schedule: eligibility-driven spread;
            # m-tiles of q-chunk c are ready at iteration 2c+3 (both its
            # units norm'd), leaving only m15 after the loop.
            OSCHED = {3: [0, 1], 4: [2, 3], 5: [4, 5], 6: [6, 7],
                      7: [8, 9], 8: [10, 11], 9: [12, 13, 14]}

            # Unit 0 runs with a deep ctx lag: its exp stream starts as soon
            # as xq chunk 0 + kT are ready (~8us before xv finishes), and
            # the V projection tiles are hooked in just after xv lands, each
            # one kt slot ahead of the ctx matmul that consumes it.
            LAG0 = min(4, ktk)

            states = []
            qparts = {}
            oparts = {}
            emit_qproj(0, 0)
            emit_qproj(0, 1)
            for i, (q0, w, p) in enumerate(units):
                # Auxiliary PE work (previous unit's normalization broadcast,
                # O-proj m-tiles, next q-chunk projection halves) is injected
                # one item per kt slot INSIDE the attention stream, so the
                # score matmuls — and with them the ACT exp stream, the
                # kernel's pacing engine — never see a multi-us convoy of
                # other PE work.
                lag = LAG0 if i == 0 else 2
                st = {"unit": (q0, w, p), "pts": [],
                      "pcA": pa.tile([65, QC], F32, tag="pcA", bufs=1, name="pcA"),
                      "pcB": pa.tile([65, QC], F32, tag="pcB", bufs=1, name="pcB"),
                      "rec": None}
                if i == 0:
                    work = [
                        (lambda m=m: emit_vproj(m)) for m in range(ktk)
                    ][::-1]
                else:
                    items = [lambda j=i - 1: emit_norm(states[j])]
                    for m in OSCHED.get(i, []):
                        items.append(lambda m=m: emit_final([m]))
                    if i in (1, 3, 5):
                        qc = i // 2 + 1
                        items.append(lambda qc=qc: emit_qproj(qc, 0))
                        items.append(lambda qc=qc: emit_qproj(qc, 1))
                    work = list(reversed(items))
                states.append(st)

                ioff = LAG0 - 1 if i == 0 else 2
                for kt in range(ktk + lag):
                    if kt < ktk:
                        emit_score(st, kt)
                    if kt >= ioff and work and kt < ktk + lag - 1:
                        work.pop()()
                    if kt >= lag:
                        emit_ctx(st, kt - lag)
                while work:
                    work.pop()()
                emit_recips(st)
            emit_norm(states[-1])
            emit_final([15])

    nc.compile()
    return nc


def _const_rows():
    cst = np.zeros((3, 128), np.float32)
    cst[0, :] = 1.0
    cst[1, 0:64] = 1.0
    cst[2, 64:128] = 1.0
    return cst


def make_in_maps(query, key, value, mask, Wq, bq, Wk, bk, Wv, bv, Wo, bo):
    """Returns (in_maps, ktk). Key positions with mask=True are dropped."""
    query = np.asarray(query, np.float32)
    key = np.asarray(key, np.float32)
    value = np.asarray(value, np.float32)
    mask = np.asarray(mask)
    Wq = np.asarray(Wq, np.float32)
    Wk = np.asarray(Wk, np.float32)
    Wv = np.asarray(Wv, np.float32)
    Wo = np.asarray(Wo, np.float32)
    bq = np.asarray(bq, np.float32)
    bk = np.asarray(bk, np.float32)
    bv = np.asarray(bv, np.float32)

    keep = [np.flatnonzero(~mask[b, 0]) for b in range(B)]
    ktk = max(1, max((len(k) + 127) // 128 for k in keep))
    SKc = 128 * ktk
    ndt = _np_dt()
    if KF8:
        idt = mybir.dt.np(FP8)

        def _q8(a):
            return np.clip(a, -240.0, 240.0).astype(idt)
    else:
        idt = ndt
        _q8 = None

    def _prearrange(w):
        # [D, n] -> [128, KT_D, n] matching the SBUF tile layout
        n = w.shape[1]
        return np.ascontiguousarray(
            w.reshape(KT_D, 128, n).transpose(1, 0, 2)
        )

    in_maps = []
    for c in range(NCORES):
        b, g = c // G, c % G
        cs = slice(g * DG, (g + 1) * DG)
        idx = keep[b]
        nk = len(idx)
        xkc = np.zeros((D, SKc), np.float32)
        xvc = np.zeros((D, SKc), np.float32)
        xkc[:, :nk] = key[b].T[:, idx]
        xvc[:, :nk] = value[b].T[:, idx]
        mbias = np.full(SKc, MASK_NEG, np.float32)
        mbias[:nk] = 0.0

        wv_aug = np.zeros((D, VW), np.float32)
        wvb_row = np.zeros((1, VW), np.float32)
        for j in range(HPG):
            src = slice(g * DG + j * DK, g * DG + (j + 1) * DK)
            wv_aug[:, j * 65 : j * 65 + 64] = Wv[:, src]
            wvb_row[0, j * 65 : j * 65 + 64] = bv[src]
            wvb_row[0, j * 65 + 64] = 1.0

        # wo: [DG, D] -> [128, NT, D]
        wo_pre = np.ascontiguousarray(
            Wo[cs, :].reshape(NT, 128, D).transpose(1, 0, 2)
        )

        xq_c = np.ascontiguousarray(query[b].T)
        wq_c = _prearrange(Wq[:, cs])
        wk_c = _prearrange(Wk[:, cs])
        if KF8:
            xq_m, xk_m = _q8(xq_c), _q8(xkc)
            wq_m, wk_m = _q8(wq_c * WSC), _q8(wk_c * WSC)
        else:
            xq_m, xk_m = xq_c.astype(ndt), xkc.astype(ndt)
            wq_m, wk_m = wq_c.astype(ndt), wk_c.astype(ndt)

        in_maps.append(
            {
                "xq": xq_m,
                "xk": xk_m,
                "xv": xvc.astype(ndt),
                "wq": wq_m,
                "wk": wk_m,
                "wv": _prearrange(wv_aug).astype(ndt),
                "wvb": wvb_row.astype(ndt),
                "wo": wo_pre.astype(ndt),
                "bq": np.ascontiguousarray(bq[cs].reshape(NT, 128).T),
                "bk": np.ascontiguousarray(bk[cs].reshape(NT, 128).T),
                "mb": np.ascontiguousarray(mbias.reshape(ktk, 128).T),
                "cstc": _const_rows().astype(ndt),
            }
        )
    return in_maps, ktk


def combine_outputs(results, mask, bo):
    mask = np.asarray(mask)
    bo = np.asarray(bo, np.float32)
    out = np.zeros((B, S, D), np.float32)
    for c in range(NCORES):
        out[c // G] += np.asarray(results[c]["out"], np.float32)
    for b in range(B):
        if mask[b, 0].all():
            # reference: fully-masked rows produce zero context
            out[b] = 0.0
    out += bo[None, None, :]
    return out


_NC_CACHE = {}


def kernel(query, key, value, mask, Wq, bq, Wk, bk, Wv, bv, Wo, bo):
    from concourse.bass_utils import run_bass_kernel_spmd

    in_maps, ktk = make_in_maps(
        query, key, value, mask, Wq, bq, Wk, bk, Wv, bv, Wo, bo
    )
    nc = _NC_CACHE.get((KDT, KF8, ktk))
    if nc is None:
        nc = _NC_CACHE[(KDT, KF8, ktk)] = build_bass(ktk)
    res = run_bass_kernel_spmd(nc, in_maps, list(range(NCORES))).results
    return combine_outputs(res, mask, bo)
